# revision 35
# baseline (speedup 1.0000x reference)
"""Trainium2 Bass kernel for nn_AttentionHead.

Computation (per batch b):
    Q = Wq @ x_b, K = Wk @ x_b, V = Wv @ x_b        (x_b: [C=256, N=4096])
    S = Q^T K   [N, N];  A = softmax_k(S)
    out_b = V @ A^T                                  ([VC=128, N])

Sharding: 8 cores = 4 batches x 2 query-halves. Each core computes K/V^T for
its full batch and Q for its 2048-query half; a flash-style loop over 32 key
chunks of 128 never materializes the full [4096, 4096] affinity.

Design notes:
 - x, weights, Q, K in float16 (10-bit mantissa keeps QK logits accurate;
   all 16-bit matmuls run at full PE rate with fast weight loads); exp
   tiles, V^T, and both outputs in bf16 (exp magnitudes exceed f16 range).
   Host casts once; input DMA is 2MB/core instead of 6MB.
 - xq is a slice of xk, so it is not shipped separately. Instead the host
   supplies xk with the core's query half first (key order permuted; softmax
   and the value aggregation are permutation-invariant over keys), and the Q
   projection reads xk columns 0:2048.
 - V^T blocks are packed 4-per-PSUM-tile so one [128,512] cast covers 4
   blocks instead of 4 separate [128,128] casts.
 - ~24 junk matmuls on the weight tile bridge the gap between the weight
   arrival and the first x chunk so the PE's HAM clock-gate releases
   (1.2 -> 2.4 GHz) before the real stream starts.
 - Softmax denominators: exp tiles are tree-summed pairwise down to one
   [128, QT] partial per query tile; half the level-0 adds run on the
   (otherwise idle) Pool engine, and the counter's tail is merged eagerly
   two iterations early so only 3 serial adds follow the last exp. Final
   128-way reduction + normalization happen on the host during unshard.
"""

import numpy as np

B, C, VC, H, W = 4, 256, 128, 64, 64
N = H * W            # keys per batch
MQ = N // 2          # queries per core
QT = 1024            # query tile (PSUM-sized)
KC = N // 128        # key chunks of 128

_cached_nc = None


def _build():
    from contextlib import ExitStack

    import concourse.bacc as bacc
    import concourse.mybir as mybir
    import concourse.tile as tile

    f32 = mybir.dt.float32
    bf16 = mybir.dt.bfloat16
    f16 = mybir.dt.float16
    Exp = mybir.ActivationFunctionType.Exp

    nc = bacc.Bacc("TRN2", target_bir_lowering=False, debug=False, num_devices=8)

    xk_d = nc.dram_tensor("xk", [C, N], f16, kind="ExternalInput")
    # all six [128, VC] weight chunks packed side by side: one DMA dispatch
    w_d = nc.dram_tensor("w", [128, 6 * VC], f16, kind="ExternalInput")
    oc_d = nc.dram_tensor("oc", [2, 128, QT], bf16, kind="ExternalOutput")
    oss_d = nc.dram_tensor("oss", [2, 128, QT], bf16, kind="ExternalOutput")

    with tile.TileContext(nc) as tc, ExitStack() as ctx:
        persist = ctx.enter_context(tc.tile_pool(name="persist", bufs=1))
        wpool = ctx.enter_context(tc.tile_pool(name="w", bufs=1))
        xp = ctx.enter_context(tc.tile_pool(name="xp", bufs=1))

        wt_all = wpool.tile([128, 6 * VC], f16, tag="w")
        nc.scalar.dma_start(wt_all[:], w_d[:, :])
        wts = {}
        for wi, nm in enumerate(("wq", "wk", "wv")):
            for cc in range(2):
                k = wi * 2 + cc
                wts[(nm, cc)] = wt_all[:, k * VC : (k + 1) * VC]

        K_t = persist.tile([128, N], f16, tag="K")
        Q_t = persist.tile([128, MQ], f16, tag="Q")
        VT = persist.tile([128, KC * 128], bf16, tag="VT")

        xk_t = [
            xp.tile([128, N], f16, tag=f"xk{cc}", name=f"xk{cc}") for cc in range(2)
        ]
        # Each 128-partition half of x streams its 4 chunks in consumption
        # order on its own queue (per-queue BW is ~110GB/s, packet-overhead
        # bound); the weights ride the third queue in parallel.
        for t in range(4):
            cs = slice(t * 1024, (t + 1) * 1024)
            nc.sync.dma_start(xk_t[0][:, cs], xk_d[0:128, cs])
            nc.gpsimd.dma_start(xk_t[1][:, cs], xk_d[128:256, cs])

        # Preload the Exp activation table while DMA streams in, so the
        # ~1.3us ACT_TABLE_LOAD is off the critical path of the first tile.
        scpool = ctx.enter_context(tc.tile_pool(name="sc", bufs=1))
        sc_in = scpool.tile([128, 8], f32, tag="sci")
        sc_out = scpool.tile([128, 8], bf16, tag="sco")
        nc.vector.memset(sc_in[:], 0.0)
        nc.scalar.activation(sc_out[:], sc_in[:], Exp)
        # DVE-memset scratch for PE warm-up matmuls: available ~5us before
        # the first DMA'd tensor, so the HAM clock-gate work starts early.
        warm_t = scpool.tile([128, 128], f16, tag="warm")
        nc.vector.memset(warm_t[:], 0.125)

        def emit_proj_tile(pool, dst, wnm, t):
            ps = pool.tile([128, 512], f32, tag="projps", name="ps")
            for cc in range(2):
                nc.tensor.matmul(
                    ps[:],
                    wts[(wnm, cc)][:],
                    xk_t[cc][:, t * 512 : (t + 1) * 512],
                    start=(cc == 0),
                    stop=(cc == 1),
                )
            nc.vector.tensor_copy(dst[:, t * 512 : (t + 1) * 512], ps[:])

        def emit_vt_group(pool, g):
            # V^T blocks 4g..4g+3: [n-block, d] = x_block.T @ Wv.T, four
            # 128-col blocks packed into one PSUM tile -> one 512-col cast.
            tp = pool.tile([128, 512], f32, tag="projps", name="tp")
            for jj in range(4):
                j = 4 * g + jj
                for cc in range(2):
                    nc.tensor.matmul(
                        tp[:, jj * 128 : (jj + 1) * 128],
                        xk_t[cc][:, j * 128 : (j + 1) * 128],
                        wts[("wv", cc)][:],
                        start=(cc == 0),
                        stop=(cc == 1),
                    )
            nc.vector.tensor_copy(VT[:, g * 512 : (g + 1) * 512], tp[:])

        # DMA-arrival floors (ms, scheduler sim-time): chunk t of x lands
        # around these times. These only shape instruction issue order.
        A = [0.0040, 0.0050, 0.0060, 0.0070]

        def chunk_floor(col_tile):          # col_tile in units of 512 cols
            return A[col_tile // 2]

        spool = ctx.enter_context(tc.tile_pool(name="spool", bufs=2, space="PSUM"))
        pcpool = ctx.enter_context(tc.tile_pool(name="pcpool", bufs=1, space="PSUM"))
        # pps holds the projection PSUM banks only while projections are
        # still being emitted (through iteration 18); closing it mid-loop
        # frees 2 banks for a second pc pool so qt=1's PV accumulation does
        # not wait on qt=0's numerator copies.
        pps_stack = ExitStack()
        pps = pps_stack.enter_context(tc.tile_pool(name="pps", bufs=2, space="PSUM"))

        # Junk matmuls on the memset scratch keep the PE busy from ~7.5us
        # (right after the engine preamble) until the first x chunk lands
        # (~12.5-13us), so the HAM clock-gate releases (1.2 -> 2.4 GHz) and
        # the projections run at full clock. Sized so even a late x arrival
        # leaves an idle gap under the ~3.4us re-throttle window.
        junk = pps.tile([128, 512], f32, tag="projps", name="junk")
        for _ in range(52):
            nc.tensor.matmul(
                junk[:, 0:128],
                warm_t[:],
                warm_t[:],
                start=True,
                stop=True,
                skip_group_check=True,
            )

        with tc.tile_wait_until(A[0]):
            emit_proj_tile(pps, Q_t, "wq", 0)
            emit_proj_tile(pps, Q_t, "wq", 1)
            emit_proj_tile(pps, K_t, "wk", 0)

        # in-loop interleave: (iteration i) -> ("k", t) or ("vt", g) or ("q", t)
        # VT group 0 sits at i=0 (not before the loop) so the first QK -> exp
        # chain isn't delayed by its 8 matmuls; PV(0) waits on exp(0) anyway.
        interleave = {0: ("vt", 0)}
        for t in range(1, 8):               # K tiles 1..7
            interleave[2 * t - 1] = ("k", t)
        for g in range(1, 8):               # VT groups 1..7
            interleave[2 * g] = ("vt", g)
        interleave[16] = ("q", 2)           # Q cols 1024:2048, needed at qt=1
        interleave[18] = ("q", 3)
        Q_LATE = 0.011

        with (
            tc.tile_pool(name="epool", bufs=8) as epool,
            tc.tile_pool(name="treep", bufs=3) as treep,
            tc.tile_pool(name="opool", bufs=2) as opool,
        ):
            pairs = [(qt, j) for qt in range(2) for j in range(KC)]
            ps_tiles = {}

            def emit_qk(qt, j):
                ps = spool.tile([128, QT], f32, tag="ps", name="ps")
                for qq in range(2):
                    nc.tensor.matmul(
                        ps[:, qq * 512 : (qq + 1) * 512],
                        K_t[:, j * 128 : (j + 1) * 128],
                        Q_t[:, qt * QT + qq * 512 : qt * QT + (qq + 1) * 512],
                        start=True,
                        stop=True,
                    )
                ps_tiles[(qt, j)] = ps

            # binary-counter pairwise reduction of exp tiles; alternate
            # level-0 adds go to the Pool engine (gpsimd) to unload DVE.
            pending = []
            l0_count = [0]

            def tree_push(t, level=0):
                while pending and pending[-1][0] == level:
                    _, other = pending.pop()
                    nt = treep.tile(
                        [128, QT], bf16, tag=f"l{level + 1}", name=f"tl{level + 1}"
                    )
                    if level == 0:
                        eng = nc.gpsimd if l0_count[0] % 2 == 0 else nc.vector
                        l0_count[0] += 1
                    else:
                        eng = nc.vector
                    eng.tensor_add(nt[:], other[:], t[:])
                    t, level = nt, level + 1
                pending.append((level, t))

            pc = None
            pcpool2 = None
            emit_qk(*pairs[0])
            for i, (qt, j) in enumerate(pairs):
                if i == 19:
                    # all projections emitted; hand pps's banks to pc2
                    pps_stack.close()
                    pcpool2 = ctx.enter_context(
                        tc.tile_pool(name="pc2", bufs=1, space="PSUM")
                    )
                if i + 1 < len(pairs):
                    emit_qk(*pairs[i + 1])
                task = interleave.get(i)
                if task is not None:
                    kind, t = task
                    if kind == "k":
                        with tc.tile_wait_until(chunk_floor(t)):
                            emit_proj_tile(pps, K_t, "wk", t)
                    elif kind == "vt":
                        with tc.tile_wait_until(A[min(t // 2, 3)]):
                            emit_vt_group(pps, t)
                    else:
                        with tc.tile_wait_until(Q_LATE):
                            emit_proj_tile(pps, Q_t, "wq", t)
                if j == 0:
                    pool = pcpool if qt == 0 else pcpool2
                    pc = pool.tile([128, QT], f32, tag="pc", name="pc")
                ps = ps_tiles.pop((qt, j))
                es = epool.tile([128, QT], bf16, tag="es", name="es")
                nc.scalar.activation(es[:], ps[:], Exp)
                first, last = j == 0, j == KC - 1
                for qq in range(2):
                    sl = slice(qq * 512, (qq + 1) * 512)
                    nc.tensor.matmul(
                        pc[:, sl],
                        VT[:, j * 128 : (j + 1) * 128],
                        es[:, sl],
                        start=first,
                        stop=last,
                    )
                tree_push(es)
                if j == KC - 3:
                    # eager tail merge: collapse the four counter levels
                    # (16+8+4+2 tiles) into two partials now, so only 3
                    # serial adds remain after the final exp instead of 5.
                    (l4, t4), (l3, t3), (l2, t2), (l1, t1) = pending
                    ea = treep.tile([128, QT], bf16, tag="ea", name="ea")
                    nc.vector.tensor_add(ea[:], t1[:], t2[:])
                    eb = treep.tile([128, QT], bf16, tag="eb", name="eb")
                    nc.vector.tensor_add(eb[:], t3[:], t4[:])
                    pending[:] = [(9, eb), (8, ea)]
                if last:
                    # numerator copies first: they free the single pc PSUM
                    # buffer so the next query-tile's PV isn't head-of-line
                    # blocked behind the denominator cascade on DVE.
                    so = opool.tile([128, QT], bf16, tag="so", name="so")
                    for qq in range(2):
                        sl = slice(qq * 512, (qq + 1) * 512)
                        # qt=1 copies go on the scalar engine, which has
                        # finished its exps by then; DVE is still running
                        # the final tree cascade.
                        if qt == 1:
                            nc.scalar.copy(so[:, sl], pc[:, sl])
                        else:
                            nc.vector.tensor_copy(so[:, sl], pc[:, sl])
                        nc.sync.dma_start(oc_d[qt, :, sl], so[:, sl])
                    acc = pending.pop()[1]
                    while pending:
                        _, t_rem = pending.pop()
                        nacc = treep.tile([128, QT], bf16, tag="fin", name="fin")
                        nc.vector.tensor_add(nacc[:], acc[:], t_rem[:])
                        acc = nacc
                    pending.clear()
                    for qq in range(2):
                        sl = slice(qq * 512, (qq + 1) * 512)
                        # scalar engine is idle at the tail (exps done);
                        # gpsimd is still busy with tree adds.
                        nc.scalar.dma_start(oss_d[qt, :, sl], acc[:, sl])

    nc.compile()
    return nc


def make_in_maps(x, Wq, Wk, Wv):
    hf = np.float16
    x = np.asarray(x, dtype=np.float32).reshape(B, C, N).astype(hf)
    # [128, 6*VC]: chunk (nm, cc) at columns (2*nm+cc)*VC is W_nm^T[cc*128:(cc+1)*128, :]
    wcat = np.concatenate(
        [
            np.asarray(Wm, dtype=np.float32).T[cc * 128 : (cc + 1) * 128, :]
            for Wm in (Wq, Wk, Wv)
            for cc in range(2)
        ],
        axis=1,
    ).astype(hf)
    wcat = np.ascontiguousarray(wcat)

    in_maps = []
    for core in range(8):
        b, h = core // 2, core % 2
        if h == 0:
            xk = np.ascontiguousarray(x[b])
        else:
            # query half first; key order is permuted identically for K and
            # V so the softmax-weighted aggregation is unchanged.
            xk = np.ascontiguousarray(
                np.concatenate([x[b][:, MQ:], x[b][:, :MQ]], axis=1)
            )
        in_maps.append({"xk": xk, "w": wcat})
    return in_maps


def assemble_output(results):
    out = np.empty((B, VC, N), dtype=np.float32)
    for core, r in enumerate(results):
        b, h = core // 2, core % 2
        sums = r["oss"].astype(np.float32).sum(axis=1, keepdims=True)  # [2,1,QT]
        core_out = r["oc"].astype(np.float32) / sums                   # [2,128,QT]
        out[b, :, h * MQ : (h + 1) * MQ] = np.concatenate(
            [core_out[0], core_out[1]], axis=1
        )
    return out.reshape(B, VC, H, W)


def _results_sane(results):
    for r in results:
        oc = np.asarray(r["oc"], dtype=np.float32)
        oss = np.asarray(r["oss"], dtype=np.float32)
        if not (np.isfinite(oc).all() and np.isfinite(oss).all()):
            return False
        if oss.sum(axis=1).min() <= 0.0:      # softmax denominators
            return False
    return True


def kernel(x, Wq, Wk, Wv):
    global _cached_nc
    from concourse.bass_utils import run_bass_kernel_spmd

    if _cached_nc is None:
        _cached_nc = _build()
    in_maps = make_in_maps(x, Wq, Wk, Wv)
    results = None
    for attempt in range(3):
        try:
            res = run_bass_kernel_spmd(
                _cached_nc, in_maps, core_ids=list(range(8))
            )
        except Exception:
            if attempt == 2:
                raise
            continue
        results = res.results
        if _results_sane(results):
            break
    return assemble_output(results)


# revision 36
# speedup vs baseline: 1.0218x; 1.0218x over previous
"""Trainium2 Bass kernel for nn_AttentionHead.

Computation (per batch b):
    Q = Wq @ x_b, K = Wk @ x_b, V = Wv @ x_b        (x_b: [C=256, N=4096])
    S = Q^T K   [N, N];  A = softmax_k(S)
    out_b = V @ A^T                                  ([VC=128, N])

Sharding: 8 cores = 4 batches x 2 query-halves. Each core computes K/V^T for
its full batch and Q for its 2048-query half; a flash-style loop over 32 key
chunks of 128 never materializes the full [4096, 4096] affinity.

Design notes:
 - x, weights, Q, K in float16 (10-bit mantissa keeps QK logits accurate;
   all 16-bit matmuls run at full PE rate with fast weight loads); exp
   tiles, V^T, and both outputs in bf16 (exp magnitudes exceed f16 range).
   Host casts once; input DMA is 2MB/core instead of 6MB.
 - xq is a slice of xk, so it is not shipped separately. Instead the host
   supplies xk with the core's query half first (key order permuted; softmax
   and the value aggregation are permutation-invariant over keys), and the Q
   projection reads xk columns 0:2048.
 - V^T blocks are packed 4-per-PSUM-tile so one [128,512] cast covers 4
   blocks instead of 4 separate [128,128] casts.
 - ~24 junk matmuls on the weight tile bridge the gap between the weight
   arrival and the first x chunk so the PE's HAM clock-gate releases
   (1.2 -> 2.4 GHz) before the real stream starts.
 - Softmax denominators: exp tiles are tree-summed pairwise down to one
   [128, QT] partial per query tile; half the level-0 adds run on the
   (otherwise idle) Pool engine, and the counter's tail is merged eagerly
   two iterations early so only 3 serial adds follow the last exp. Final
   128-way reduction + normalization happen on the host during unshard.
"""

import numpy as np

B, C, VC, H, W = 4, 256, 128, 64, 64
N = H * W            # keys per batch
MQ = N // 2          # queries per core
QT = 1024            # query tile (PSUM-sized)
KC = N // 128        # key chunks of 128

_cached_nc = None


def _build():
    from contextlib import ExitStack

    import concourse.bacc as bacc
    import concourse.mybir as mybir
    import concourse.tile as tile

    f32 = mybir.dt.float32
    bf16 = mybir.dt.bfloat16
    f16 = mybir.dt.float16
    Exp = mybir.ActivationFunctionType.Exp

    nc = bacc.Bacc("TRN2", target_bir_lowering=False, debug=False, num_devices=8)

    xk_d = nc.dram_tensor("xk", [C, N], f16, kind="ExternalInput")
    # all six [128, VC] weight chunks packed side by side: one DMA dispatch
    w_d = nc.dram_tensor("w", [128, 6 * VC], f16, kind="ExternalInput")
    oc_d = nc.dram_tensor("oc", [2, 128, QT], bf16, kind="ExternalOutput")
    oss_d = nc.dram_tensor("oss", [2, 128, QT], bf16, kind="ExternalOutput")

    with tile.TileContext(nc) as tc, ExitStack() as ctx:
        persist = ctx.enter_context(tc.tile_pool(name="persist", bufs=1))
        wpool = ctx.enter_context(tc.tile_pool(name="w", bufs=1))
        xp = ctx.enter_context(tc.tile_pool(name="xp", bufs=1))

        wt_all = wpool.tile([128, 6 * VC], f16, tag="w")
        nc.scalar.dma_start(wt_all[:], w_d[:, :])
        wts = {}
        for wi, nm in enumerate(("wq", "wk", "wv")):
            for cc in range(2):
                k = wi * 2 + cc
                wts[(nm, cc)] = wt_all[:, k * VC : (k + 1) * VC]

        K_t = persist.tile([128, N], f16, tag="K")
        Q_t = persist.tile([128, MQ], f16, tag="Q")
        VT = persist.tile([128, KC * 128], bf16, tag="VT")

        xk_t = [
            xp.tile([128, N], f16, tag=f"xk{cc}", name=f"xk{cc}") for cc in range(2)
        ]
        # Each 128-partition half of x streams its 4 chunks in consumption
        # order on its own queue (per-queue BW is ~110GB/s, packet-overhead
        # bound); the weights ride the third queue in parallel.
        for t in range(4):
            cs = slice(t * 1024, (t + 1) * 1024)
            nc.sync.dma_start(xk_t[0][:, cs], xk_d[0:128, cs])
            nc.gpsimd.dma_start(xk_t[1][:, cs], xk_d[128:256, cs])

        # Preload the Exp activation table while DMA streams in, so the
        # ~1.3us ACT_TABLE_LOAD is off the critical path of the first tile.
        scpool = ctx.enter_context(tc.tile_pool(name="sc", bufs=1))
        sc_in = scpool.tile([128, 8], f32, tag="sci")
        sc_out = scpool.tile([128, 8], bf16, tag="sco")
        nc.vector.memset(sc_in[:], 0.0)
        nc.scalar.activation(sc_out[:], sc_in[:], Exp)
        # DVE-memset scratch for PE warm-up matmuls: available ~5us before
        # the first DMA'd tensor, so the HAM clock-gate work starts early.
        warm_t = scpool.tile([128, 128], f16, tag="warm")
        nc.vector.memset(warm_t[:], 0.125)

        def emit_proj_tile(pool, dst, wnm, t):
            ps = pool.tile([128, 512], f32, tag="projps", name="ps")
            for cc in range(2):
                nc.tensor.matmul(
                    ps[:],
                    wts[(wnm, cc)][:],
                    xk_t[cc][:, t * 512 : (t + 1) * 512],
                    start=(cc == 0),
                    stop=(cc == 1),
                )
            nc.vector.tensor_copy(dst[:, t * 512 : (t + 1) * 512], ps[:])

        def emit_vt_group(pool, g):
            # V^T blocks 4g..4g+3: [n-block, d] = x_block.T @ Wv.T, four
            # 128-col blocks packed into one PSUM tile -> one 512-col cast.
            tp = pool.tile([128, 512], f32, tag="projps", name="tp")
            for jj in range(4):
                j = 4 * g + jj
                for cc in range(2):
                    nc.tensor.matmul(
                        tp[:, jj * 128 : (jj + 1) * 128],
                        xk_t[cc][:, j * 128 : (j + 1) * 128],
                        wts[("wv", cc)][:],
                        start=(cc == 0),
                        stop=(cc == 1),
                    )
            nc.vector.tensor_copy(VT[:, g * 512 : (g + 1) * 512], tp[:])

        # DMA-arrival floors (ms, scheduler sim-time): chunk t of x lands
        # around these times. These only shape instruction issue order.
        A = [0.0040, 0.0050, 0.0060, 0.0070]

        def chunk_floor(col_tile):          # col_tile in units of 512 cols
            return A[col_tile // 2]

        spool = ctx.enter_context(tc.tile_pool(name="spool", bufs=2, space="PSUM"))
        pcpool = ctx.enter_context(tc.tile_pool(name="pcpool", bufs=1, space="PSUM"))
        # pps holds the projection PSUM banks only while projections are
        # still being emitted (through iteration 18); closing it mid-loop
        # frees 2 banks for a second pc pool so qt=1's PV accumulation does
        # not wait on qt=0's numerator copies.
        pps_stack = ExitStack()
        pps = pps_stack.enter_context(tc.tile_pool(name="pps", bufs=2, space="PSUM"))

        # Junk matmuls on the memset scratch keep the PE busy from ~7.5us
        # (right after the engine preamble) until the first x chunk lands
        # (~12.5-13us), so the HAM clock-gate releases (1.2 -> 2.4 GHz) and
        # the projections run at full clock. Sized so even a late x arrival
        # leaves an idle gap under the ~3.4us re-throttle window.
        junk = pps.tile([128, 512], f32, tag="projps", name="junk")
        for _ in range(52):
            nc.tensor.matmul(
                junk[:, 0:128],
                warm_t[:],
                warm_t[:],
                start=True,
                stop=True,
                skip_group_check=True,
            )

        with tc.tile_wait_until(A[0]):
            emit_proj_tile(pps, Q_t, "wq", 0)
            emit_proj_tile(pps, Q_t, "wq", 1)
            emit_proj_tile(pps, K_t, "wk", 0)

        # in-loop interleave: (iteration i) -> ("k", t) or ("vt", g) or ("q", t)
        # VT group 0 sits at i=0 (not before the loop) so the first QK -> exp
        # chain isn't delayed by its 8 matmuls; PV(0) waits on exp(0) anyway.
        interleave = {0: ("vt", 0)}
        for t in range(1, 8):               # K tiles 1..7
            interleave[2 * t - 1] = ("k", t)
        for g in range(1, 8):               # VT groups 1..7
            interleave[2 * g] = ("vt", g)
        interleave[16] = ("q", 2)           # Q cols 1024:2048, needed at qt=1
        interleave[18] = ("q", 3)
        Q_LATE = 0.011

        with (
            tc.tile_pool(name="epool", bufs=8) as epool,
            tc.tile_pool(name="treep", bufs=3) as treep,
            tc.tile_pool(name="opool", bufs=2) as opool,
        ):
            pairs = [(qt, j) for qt in range(2) for j in range(KC)]
            ps_tiles = {}
            spool2 = [None]
            qk_count = [0]

            def emit_qk(qt, j):
                qk_count[0] += 1
                if spool2[0] is not None and qk_count[0] % 3 == 0:
                    ps = spool2[0].tile([128, QT], f32, tag="ps2", name="ps2")
                else:
                    ps = spool.tile([128, QT], f32, tag="ps", name="ps")
                for qq in range(2):
                    nc.tensor.matmul(
                        ps[:, qq * 512 : (qq + 1) * 512],
                        K_t[:, j * 128 : (j + 1) * 128],
                        Q_t[:, qt * QT + qq * 512 : qt * QT + (qq + 1) * 512],
                        start=True,
                        stop=True,
                    )
                ps_tiles[(qt, j)] = ps

            # binary-counter pairwise reduction of exp tiles; alternate
            # level-0 adds go to the Pool engine (gpsimd) to unload DVE.
            pending = []
            l0_count = [0]

            def tree_push(t, level=0):
                while pending and pending[-1][0] == level:
                    _, other = pending.pop()
                    nt = treep.tile(
                        [128, QT], bf16, tag=f"l{level + 1}", name=f"tl{level + 1}"
                    )
                    if level == 0:
                        eng = nc.gpsimd if l0_count[0] % 2 == 0 else nc.vector
                        l0_count[0] += 1
                    else:
                        eng = nc.vector
                    eng.tensor_add(nt[:], other[:], t[:])
                    t, level = nt, level + 1
                pending.append((level, t))

            pc = None
            emit_qk(*pairs[0])
            for i, (qt, j) in enumerate(pairs):
                if i == 19:
                    # all projections emitted; hand pps's banks to a third
                    # S-tile buffer so the exp stream can run two tiles
                    # ahead of the PE (rides out PV head-of-line stalls at
                    # the query-tile transition).
                    pps_stack.close()
                    spool2[0] = ctx.enter_context(
                        tc.tile_pool(name="spool2", bufs=1, space="PSUM")
                    )
                if i + 1 < len(pairs):
                    emit_qk(*pairs[i + 1])
                task = interleave.get(i)
                if task is not None:
                    kind, t = task
                    if kind == "k":
                        with tc.tile_wait_until(chunk_floor(t)):
                            emit_proj_tile(pps, K_t, "wk", t)
                    elif kind == "vt":
                        with tc.tile_wait_until(A[min(t // 2, 3)]):
                            emit_vt_group(pps, t)
                    else:
                        with tc.tile_wait_until(Q_LATE):
                            emit_proj_tile(pps, Q_t, "wq", t)
                if j == 0:
                    pc = pcpool.tile([128, QT], f32, tag="pc", name="pc")
                ps = ps_tiles.pop((qt, j))
                es = epool.tile([128, QT], bf16, tag="es", name="es")
                nc.scalar.activation(es[:], ps[:], Exp)
                first, last = j == 0, j == KC - 1
                for qq in range(2):
                    sl = slice(qq * 512, (qq + 1) * 512)
                    nc.tensor.matmul(
                        pc[:, sl],
                        VT[:, j * 128 : (j + 1) * 128],
                        es[:, sl],
                        start=first,
                        stop=last,
                    )
                tree_push(es)
                if j == KC - 3:
                    # eager tail merge: collapse the four counter levels
                    # (16+8+4+2 tiles) into two partials now, so only 3
                    # serial adds remain after the final exp instead of 5.
                    (l4, t4), (l3, t3), (l2, t2), (l1, t1) = pending
                    ea = treep.tile([128, QT], bf16, tag="ea", name="ea")
                    nc.vector.tensor_add(ea[:], t1[:], t2[:])
                    eb = treep.tile([128, QT], bf16, tag="eb", name="eb")
                    nc.vector.tensor_add(eb[:], t3[:], t4[:])
                    pending[:] = [(9, eb), (8, ea)]
                if last:
                    # numerator copies first: they free the single pc PSUM
                    # buffer so the next query-tile's PV isn't head-of-line
                    # blocked behind the denominator cascade on DVE.
                    so = opool.tile([128, QT], bf16, tag="so", name="so")
                    for qq in range(2):
                        sl = slice(qq * 512, (qq + 1) * 512)
                        # qt=1 copies go on the scalar engine, which has
                        # finished its exps by then; DVE is still running
                        # the final tree cascade.
                        if qt == 1:
                            nc.scalar.copy(so[:, sl], pc[:, sl])
                        else:
                            nc.vector.tensor_copy(so[:, sl], pc[:, sl])
                        nc.sync.dma_start(oc_d[qt, :, sl], so[:, sl])
                    acc = pending.pop()[1]
                    while pending:
                        _, t_rem = pending.pop()
                        nacc = treep.tile([128, QT], bf16, tag="fin", name="fin")
                        nc.vector.tensor_add(nacc[:], acc[:], t_rem[:])
                        acc = nacc
                    pending.clear()
                    for qq in range(2):
                        sl = slice(qq * 512, (qq + 1) * 512)
                        # scalar engine is idle at the tail (exps done);
                        # gpsimd is still busy with tree adds.
                        nc.scalar.dma_start(oss_d[qt, :, sl], acc[:, sl])

    nc.compile()
    return nc


def make_in_maps(x, Wq, Wk, Wv):
    hf = np.float16
    x = np.asarray(x, dtype=np.float32).reshape(B, C, N).astype(hf)
    # [128, 6*VC]: chunk (nm, cc) at columns (2*nm+cc)*VC is W_nm^T[cc*128:(cc+1)*128, :]
    wcat = np.concatenate(
        [
            np.asarray(Wm, dtype=np.float32).T[cc * 128 : (cc + 1) * 128, :]
            for Wm in (Wq, Wk, Wv)
            for cc in range(2)
        ],
        axis=1,
    ).astype(hf)
    wcat = np.ascontiguousarray(wcat)

    in_maps = []
    for core in range(8):
        b, h = core // 2, core % 2
        if h == 0:
            xk = np.ascontiguousarray(x[b])
        else:
            # query half first; key order is permuted identically for K and
            # V so the softmax-weighted aggregation is unchanged.
            xk = np.ascontiguousarray(
                np.concatenate([x[b][:, MQ:], x[b][:, :MQ]], axis=1)
            )
        in_maps.append({"xk": xk, "w": wcat})
    return in_maps


def assemble_output(results):
    out = np.empty((B, VC, N), dtype=np.float32)
    for core, r in enumerate(results):
        b, h = core // 2, core % 2
        sums = r["oss"].astype(np.float32).sum(axis=1, keepdims=True)  # [2,1,QT]
        core_out = r["oc"].astype(np.float32) / sums                   # [2,128,QT]
        out[b, :, h * MQ : (h + 1) * MQ] = np.concatenate(
            [core_out[0], core_out[1]], axis=1
        )
    return out.reshape(B, VC, H, W)


def _results_sane(results):
    for r in results:
        oc = np.asarray(r["oc"], dtype=np.float32)
        oss = np.asarray(r["oss"], dtype=np.float32)
        if not (np.isfinite(oc).all() and np.isfinite(oss).all()):
            return False
        if oss.sum(axis=1).min() <= 0.0:      # softmax denominators
            return False
    return True


def kernel(x, Wq, Wk, Wv):
    global _cached_nc
    from concourse.bass_utils import run_bass_kernel_spmd

    if _cached_nc is None:
        _cached_nc = _build()
    in_maps = make_in_maps(x, Wq, Wk, Wv)
    results = None
    for attempt in range(3):
        try:
            res = run_bass_kernel_spmd(
                _cached_nc, in_maps, core_ids=list(range(8))
            )
        except Exception:
            if attempt == 2:
                raise
            continue
        results = res.results
        if _results_sane(results):
            break
    return assemble_output(results)
